# revision 27
# baseline (speedup 1.0000x reference)
"""DynamicConv (attention-over-kernel-bank conv2d) on 8 Trainium2 NeuronCores.

Winograd F(2x2, 3x3) formulation, data-parallel over batch N=32 (4/core).

Host side: the attention path (pool -> MLP -> softmax -> pi) and the two
Winograd constant transforms are applied on host: the kernel bank becomes
U = G W G^T (shipped as delta-form for a short aggregation chain) and the
input becomes V = B^T d B per 4x4 tile (stride 2, 32x32 tile grid).

Device side, per sample:
  1. aggregate transformed kernel:  aggP[ci, uv, co] = U3 + sum_m pi_m dU_m
     (3 scalar_tensor_tensor ops, bf16; co-half 0 on DVE, half 1 on GPSIMD)
     plus negated copies of the u>=2 planes for the folded output transform.
  2. for each (grid-half, co-tile, a): accumulate the A^T-folded tiles
     t[a, v] = sum_u s_au * (U[u,v] (x) V[u,v]) as 24 matmuls into one
     4-bank PSUM tile [128, 4, 512] (8 banks rotate between 2 phases).
  3. ScalarE copies the PSUM block to SBUF f16; DVE (a=0) / GPSIMD (a=1)
     apply the right A transform + bias: y[b0] = t0+t1+t2+bn,
     y[b1] = t1-t2-t3+bn; DMA out f16, host de-interleaves quadrants.
"""

from contextlib import ExitStack

import ml_dtypes
import numpy as np

import concourse.bass as bass
import concourse.tile as tile
from concourse import bacc, bass_utils, mybir

N, CI, CO, KK, H, W, M = 32, 256, 256, 3, 64, 64, 4
TAU = 1.0 / 30.0
NCORES = 8
NL = N // NCORES          # samples per core
CIT, COT = CI // 128, CO // 128
G_T = 32                  # winograd tile grid is 32x32
GRID = G_T * G_T          # 1024 positions per sample
HALF = GRID // 2          # grid positions per phase (512)
UV = 16

F32 = mybir.dt.float32
F16 = mybir.dt.float16
BF16 = mybir.dt.bfloat16
BF16_NP = ml_dtypes.bfloat16

_CACHE: dict = {}


def _emit(ctx: ExitStack, tc: tile.TileContext):
    nc = tc.nc
    ALU = mybir.AluOpType
    AF = mybir.ActivationFunctionType

    # U in delta form: slot m<3 = U_m - U_3, slot 3 = U_3; co-half major so
    # every bank DMA and aggregation slice is fully contiguous
    wb_d = nc.dram_tensor("wb", (M, CIT, COT, 128, UV, 128), BF16, kind="ExternalInput").ap()
    v_d = nc.dram_tensor("vt", (NL, 2, CIT, 128, UV, HALF), BF16, kind="ExternalInput").ap()
    # [:, 0:16] pi broadcast (n*M+m), [:, 16:24] bnT [COT, NL]
    cst_d = nc.dram_tensor("cst", (128, 24), F32, kind="ExternalInput").ap()
    y_d = nc.dram_tensor("y", (NL, COT, 128, 2, 2, GRID), F16, kind="ExternalOutput").ap()

    consts = ctx.enter_context(tc.tile_pool(name="consts", bufs=1))
    vpool = ctx.enter_context(tc.tile_pool(name="vpool", bufs=3))
    aggp_pool = ctx.enter_context(tc.tile_pool(name="aggp", bufs=3))
    aggn_pool = ctx.enter_context(tc.tile_pool(name="aggn", bufs=3))
    tcp_pool = ctx.enter_context(tc.tile_pool(name="tcp", bufs=2))
    scr_pool = ctx.enter_context(tc.tile_pool(name="scr", bufs=2))
    outp = ctx.enter_context(tc.tile_pool(name="outp", bufs=2))
    psum = ctx.enter_context(tc.tile_pool(name="psum", bufs=2, space="PSUM"))

    cst_sb = consts.tile([128, 24], F32)
    nc.sync.dma_start(cst_sb[:], cst_d[:])
    pi_b = cst_sb[:, 0:16]
    bnT = cst_sb[:, 16:24]

    # bank in SBUF, co-half major: [128, m, cit, co-half, uv, 128co]
    wb_sb = consts.tile([128, M, CIT, COT, UV, 128], BF16)

    def wb_dma(ch, t, ms=(0, 3, 1, 2)):
        # chain order: op1 reads slots 0 and 3, then 1, then 2
        for m in ms:
            nc.sync.dma_start(wb_sb[:, m, t, ch], wb_d[m, t, ch])

    vts = {}

    def v_alloc(n, h):
        vts[(n, h)] = vpool.tile([128, CIT, UV, HALF], BF16, tag="vt", name="vt")

    def v_dma(n, h, t):
        nc.sync.dma_start(vts[(n, h)][:, t], v_d[n, h, t])

    aggs = {}

    def agg_alloc(n, ch):
        aggP = aggp_pool.tile([128, CIT, UV, 128], BF16, tag="aggP", name="aggP")
        aggN1 = aggn_pool.tile([128, CIT, 4, 128], BF16, tag="aggN1", name="aggN1")
        aggs[(n, ch)] = (aggP, aggN1)

    def agg_block(n, ch, t):
        # all on DVE: Pool rejects scalar_tensor_tensor at codegen. every
        # operand is a fully contiguous slab.
        s = n * M
        aggP, aggN1 = aggs[(n, ch)]
        o = aggP[:, t]
        nc.vector.scalar_tensor_tensor(o, wb_sb[:, 0, t, ch], pi_b[:, s : s + 1], wb_sb[:, 3, t, ch], op0=ALU.mult, op1=ALU.add)
        nc.vector.scalar_tensor_tensor(o, wb_sb[:, 1, t, ch], pi_b[:, s + 1 : s + 2], o, op0=ALU.mult, op1=ALU.add)
        nc.vector.scalar_tensor_tensor(o, wb_sb[:, 2, t, ch], pi_b[:, s + 2 : s + 3], o, op0=ALU.mult, op1=ALU.add)
        # negated u=1 planes (v-major: indices 1,5,9,13): the a=1 phase
        # accumulates q = -m1 + m2 + m3, drained with scale -1
        nc.vector.tensor_scalar_mul(aggN1[:, t], aggP[:, t, 1:16:4, :], -1.0)

    def phase(n, h, ct, a):
        vt = vts[(n, h)]
        aggP, aggN1 = aggs[(n, ct)]
        pt = psum.tile([128, 4, HALF], F32, tag="pt", name="pt")
        # plane order is v-major (p = v*4+u) so the head can stream V per
        # v-group; aggN1 holds only the u=1 planes (indexed by v)
        if a == 0:
            # t0 = m0 + m1 + m2
            terms = ((aggP, 0, lambda v: v * 4 + 0), (aggP, 1, lambda v: v * 4 + 1), (aggP, 2, lambda v: v * 4 + 2))
        else:
            # q = -m1 + m2 + m3 ; t1 = -q applied at drain time
            terms = ((aggN1, 1, lambda v: v), (aggP, 2, lambda v: v * 4 + 2), (aggP, 3, lambda v: v * 4 + 3))
        for v in range(4):
            i = 0
            for t in range(CIT):
                for wtile, u, plane in terms:
                    nc.tensor.matmul(
                        pt[:, v, :],
                        wtile[:, t, plane(v), :],
                        vt[:, t, v * 4 + u, :],
                        start=(i == 0),
                        stop=(i == 2 * len(terms) - 1),
                    )
                    i += 1
        # ScalarE drains the 4-bank PSUM block to SBUF f16 as FIVE planes
        # [t0, t1+bn, t2, -t2, -t3] (sign-flipped wholesale for a=1, where
        # the PSUM holds -t1 components). The negations live here because
        # DVE SUBTRACT has no fast-mode uop (~11x slower than ADD).
        sgn = 1.0 if a == 0 else -1.0
        bn = bnT[:, ct * NL + n : ct * NL + n + 1]
        tcp = tcp_pool.tile([128, 5, HALF], F16, tag="tcp", name="tcp")
        nc.scalar.activation(tcp[:, 0, :], pt[:, 0, :], AF.Copy, scale=sgn)
        nc.scalar.activation(tcp[:, 1, :], pt[:, 1, :], AF.Identity, bias=bn, scale=sgn)
        nc.scalar.activation(tcp[:, 2, :], pt[:, 2, :], AF.Copy, scale=sgn)
        # the negated planes ride DVE: plane 3 re-negates the f16 copy at 4x,
        # plane 4 drains PSUM directly (DVE has slack; ScalarE was binding)
        nc.vector.tensor_scalar_mul(tcp[:, 3, :], tcp[:, 2, :], -1.0)
        nc.vector.tensor_scalar_mul(tcp[:, 4, :], pt[:, 3, :], -sgn)
        # right A transform, pure 2x-mode ADDs on DVE:
        # y[b0] = (t0+t1)+t2 ; y[b1] = (t1+(-t2))+(-t3)
        ot = outp.tile([128, 2, HALF], F16, tag="ot", name="ot")
        s01 = scr_pool.tile([128, HALF], F16, tag="scr", name="scr")
        nc.vector.tensor_tensor(s01[:], tcp[:, 0, :], tcp[:, 1, :], op=ALU.add)
        nc.vector.tensor_tensor(ot[:, 0, :], s01[:], tcp[:, 2, :], op=ALU.add)
        d12 = scr_pool.tile([128, HALF], F16, tag="scr", name="scr")
        nc.vector.tensor_tensor(d12[:], tcp[:, 1, :], tcp[:, 3, :], op=ALU.add)
        nc.vector.tensor_tensor(ot[:, 1, :], d12[:], tcp[:, 4, :], op=ALU.add)
        nc.sync.dma_start(y_d[n, ct, :, a, :, h * HALF : (h + 1) * HALF], ot[:])

    # ---- head: the bytes gating the first matmuls lead, in consumption
    # order (the DMA ring completes in issue order): ch0 bank + V half 0,
    # then ch1 bank (phases 2-3), then V half 1 (phases 4-7). ----
    v_alloc(0, 0)
    v_alloc(0, 1)
    wb_dma(0, 0, ms=(0, 3))
    for g in range(2):
        for t in range(CIT):
            nc.sync.dma_start(vts[(0, 0)][:, t, 4 * g : 4 * g + 4, :], v_d[0, 0, t, :, 4 * g : 4 * g + 4, :])
    wb_dma(0, 0, ms=(1, 2))
    wb_dma(0, 1)
    for g in range(2, 4):
        for t in range(CIT):
            nc.sync.dma_start(vts[(0, 0)][:, t, 4 * g : 4 * g + 4, :], v_d[0, 0, t, :, 4 * g : 4 * g + 4, :])
    agg_alloc(0, 0)
    agg_block(0, 0, 0)
    agg_block(0, 0, 1)
    wb_dma(1, 0)
    wb_dma(1, 1)
    agg_alloc(0, 1)
    agg_block(0, 1, 0)
    agg_block(0, 1, 1)
    v_dma(0, 1, 0)
    v_dma(0, 1, 1)

    for n in range(NL):
        plan = [(h, ct, a) for h in range(2) for ct in range(COT) for a in range(2)]
        for i, (h, ct, a) in enumerate(plan):
            phase(n, h, ct, a)
            if n + 1 < NL:
                # interleave the next sample's aggregation and V prefetch at
                # the points where buffers free up (DVE and the sync queue
                # both execute in issue order)
                if i == 3:
                    v_alloc(n + 1, 0)
                    v_dma(n + 1, 0, 0)
                    v_dma(n + 1, 0, 1)
                    agg_alloc(n + 1, 0)
                    agg_block(n + 1, 0, 0)
                elif i == 4:
                    agg_block(n + 1, 0, 1)
                elif i == 5:
                    agg_alloc(n + 1, 1)
                    agg_block(n + 1, 1, 0)
                elif i == 6:
                    agg_block(n + 1, 1, 1)
                elif i == 7:
                    v_alloc(n + 1, 1)
                    v_dma(n + 1, 1, 0)
                    v_dma(n + 1, 1, 1)


def build_program():
    nc = bacc.Bacc("TRN2", target_bir_lowering=False, debug=False, num_devices=NCORES)
    with tile.TileContext(nc) as tc:
        with ExitStack() as ctx:
            _emit(ctx, tc)
    nc.compile()
    return nc


def _host_pi(x, w1, b1, w2, b2):
    pooled = x.mean(axis=(2, 3), dtype=np.float32)
    hmid = np.maximum(pooled @ np.asarray(w1, np.float32).T + np.asarray(b1, np.float32), 0)
    logits = hmid @ np.asarray(w2, np.float32).T + np.asarray(b2, np.float32)
    z = logits * TAU
    z = z - z.max(axis=1, keepdims=True)
    e = np.exp(z)
    return (e / e.sum(axis=1, keepdims=True)).astype(np.float32)


def _wino_input(x):
    """V[n, ci, uv, k, j] = (B^T d B) for 4x4 tiles of the padded input."""
    n, ci = x.shape[0], x.shape[1]
    xpad = np.zeros((n, ci, H + 2, W + 2), np.float32)
    xpad[:, :, 1 : H + 1, 1 : W + 1] = x
    e = xpad[:, :, :, 0::2]          # 33 even cols
    o = xpad[:, :, :, 1::2]          # 33 odd cols
    R = np.empty((4, n, ci, H + 2, G_T), np.float32)
    R[0] = e[:, :, :, :G_T] - e[:, :, :, 1:]
    R[1] = o[:, :, :, :G_T] + e[:, :, :, 1:]
    R[2] = e[:, :, :, 1:] - o[:, :, :, :G_T]
    R[3] = o[:, :, :, :G_T] - o[:, :, :, 1:]
    V = np.empty((n, ci, UV, G_T, G_T), np.float32)
    # plane order v-major: p = v*4 + u
    for v in range(4):
        er = R[v][:, :, 0::2, :]     # 33 even rows
        orr = R[v][:, :, 1::2, :]    # 33 odd rows
        V[:, :, v * 4 + 0] = er[:, :, :G_T] - er[:, :, 1:]
        V[:, :, v * 4 + 1] = orr[:, :, :G_T] + er[:, :, 1:]
        V[:, :, v * 4 + 2] = er[:, :, 1:] - orr[:, :, :G_T]
        V[:, :, v * 4 + 3] = orr[:, :, :G_T] - orr[:, :, 1:]
    return V


def prep_inputs(x, Wbank, Bbank, w1, b1, w2, b2):
    """Host-side layout prep. Returns per-core in_maps."""
    x = np.asarray(x, dtype=np.float32)
    Wbank = np.asarray(Wbank, dtype=np.float32)
    pi = _host_pi(x, w1, b1, w2, b2)                                   # N,M
    bn = pi @ np.asarray(Bbank, np.float32).T                          # N,CO

    # U = G W G^T per (co, m, ci); delta form over m
    G = np.array([[1, 0, 0], [0.5, 0.5, 0.5], [0.5, -0.5, 0.5], [0, 0, 1]], np.float32)
    # plane order v-major to match V: index [v, u]
    U = np.einsum("ua,omiab,vb->omivu", G, Wbank, G).astype(np.float32)  # Co,M,Ci,4v,4u
    Ud = np.empty_like(U)
    Ud[:, 3] = U[:, 3]
    for m in range(3):
        Ud[:, m] = U[:, m] - U[:, 3]
    # wb_d [m, cit, co-half, 128, uv, 128]
    wb = np.ascontiguousarray(
        Ud.reshape(COT, 128, M, CIT, 128, UV).transpose(2, 3, 0, 4, 5, 1)
    ).astype(BF16_NP)

    # V -> v_d [NL_core..., 2, cit, 128, uv, 512]
    V = _wino_input(x)                                                 # N,CI,16,32,32
    Vr = V.reshape(N, CIT, 128, UV, GRID)
    v_all = np.ascontiguousarray(
        Vr.reshape(N, CIT, 128, UV, 2, HALF).transpose(0, 4, 1, 2, 3, 5)
    ).astype(BF16_NP)                                                  # N,2,CIT,128,UV,HALF

    in_maps = []
    for c in range(NCORES):
        sl = slice(c * NL, (c + 1) * NL)
        cst = np.zeros((128, 24), dtype=np.float32)
        cst[:, 0:16] = np.broadcast_to(pi[sl].reshape(1, NL * M), (128, NL * M))
        cst[:, 16:24] = bn[sl].reshape(NL, COT, 128).transpose(2, 1, 0).reshape(128, COT * NL)
        in_maps.append({"vt": np.ascontiguousarray(v_all[sl]), "wb": wb, "cst": cst})
    return in_maps


def kernel(x, Wbank, Bbank, w1, b1, w2, b2):
    x = np.asarray(x, dtype=np.float32)
    in_maps = prep_inputs(x, Wbank, Bbank, w1, b1, w2, b2)
    if "nc" not in _CACHE:
        _CACHE["nc"] = build_program()
    res = bass_utils.run_bass_kernel_spmd(_CACHE["nc"], in_maps, core_ids=list(range(NCORES)))
    outs = []
    for r in res.results:
        y = r["y"].astype(np.float32)                                  # NL,COT,128,2,2,GRID
        y = y.reshape(NL, COT, 128, 2, 2, G_T, G_T)
        y = y.transpose(0, 1, 2, 5, 3, 6, 4).reshape(NL, CO, H, W)
        outs.append(y)
    return np.concatenate(outs, axis=0)


# revision 28
# speedup vs baseline: 1.0920x; 1.0920x over previous
"""DynamicConv (attention-over-kernel-bank conv2d) on 8 Trainium2 NeuronCores.

Winograd F(2x2, 3x3) formulation, data-parallel over batch N=32 (4/core).

Host side: the attention path (pool -> MLP -> softmax -> pi) and the two
Winograd constant transforms are applied on host: the kernel bank becomes
U = G W G^T (shipped as delta-form for a short aggregation chain) and the
input becomes V = B^T d B per 4x4 tile (stride 2, 32x32 tile grid).

Device side, per sample:
  1. aggregate transformed kernel:  aggP[ci, uv, co] = U3 + sum_m pi_m dU_m
     (3 scalar_tensor_tensor ops, bf16; co-half 0 on DVE, half 1 on GPSIMD)
     plus negated copies of the u>=2 planes for the folded output transform.
  2. for each (grid-half, co-tile, a): accumulate the A^T-folded tiles
     t[a, v] = sum_u s_au * (U[u,v] (x) V[u,v]) as 24 matmuls into one
     4-bank PSUM tile [128, 4, 512] (8 banks rotate between 2 phases).
  3. ScalarE copies the PSUM block to SBUF f16; DVE (a=0) / GPSIMD (a=1)
     apply the right A transform + bias: y[b0] = t0+t1+t2+bn,
     y[b1] = t1-t2-t3+bn; DMA out f16, host de-interleaves quadrants.
"""

from contextlib import ExitStack

import ml_dtypes
import numpy as np

import concourse.bass as bass
import concourse.tile as tile
from concourse import bacc, bass_utils, mybir

N, CI, CO, KK, H, W, M = 32, 256, 256, 3, 64, 64, 4
TAU = 1.0 / 30.0
NCORES = 8
NL = N // NCORES          # samples per core
CIT, COT = CI // 128, CO // 128
G_T = 32                  # winograd tile grid is 32x32
GRID = G_T * G_T          # 1024 positions per sample
HALF = GRID // 2          # grid positions per phase (512)
UV = 16

F32 = mybir.dt.float32
F16 = mybir.dt.float16
BF16 = mybir.dt.bfloat16
BF16_NP = ml_dtypes.bfloat16

_CACHE: dict = {}


def _emit(ctx: ExitStack, tc: tile.TileContext):
    nc = tc.nc
    ALU = mybir.AluOpType
    AF = mybir.ActivationFunctionType

    # U in delta form: slot m<3 = U_m - U_3, slot 3 = U_3; co-half major so
    # every bank DMA and aggregation slice is fully contiguous
    wb_d = nc.dram_tensor("wb", (M, CIT, COT, 128, UV, 128), BF16, kind="ExternalInput").ap()
    v_d = nc.dram_tensor("vt", (NL, 2, CIT, 128, UV, HALF), BF16, kind="ExternalInput").ap()
    # [:, 0:16] pi broadcast (n*M+m), [:, 16:24] bnT [COT, NL]
    cst_d = nc.dram_tensor("cst", (128, 24), F32, kind="ExternalInput").ap()
    y_d = nc.dram_tensor("y", (NL, COT, 128, 2, 2, GRID), F16, kind="ExternalOutput").ap()

    consts = ctx.enter_context(tc.tile_pool(name="consts", bufs=1))
    vpool = ctx.enter_context(tc.tile_pool(name="vpool", bufs=3))
    aggp_pool = ctx.enter_context(tc.tile_pool(name="aggp", bufs=3))
    aggn_pool = ctx.enter_context(tc.tile_pool(name="aggn", bufs=3))
    tcp_pool = ctx.enter_context(tc.tile_pool(name="tcp", bufs=2))
    scr_pool = ctx.enter_context(tc.tile_pool(name="scr", bufs=2))
    outp = ctx.enter_context(tc.tile_pool(name="outp", bufs=2))
    psum = ctx.enter_context(tc.tile_pool(name="psum", bufs=2, space="PSUM"))

    cst_sb = consts.tile([128, 24], F32)
    nc.sync.dma_start(cst_sb[:], cst_d[:])
    pi_b = cst_sb[:, 0:16]
    bnT = cst_sb[:, 16:24]

    # bank in SBUF, co-half major: [128, m, cit, co-half, uv, 128co]
    wb_sb = consts.tile([128, M, CIT, COT, UV, 128], BF16)

    def wb_dma(ch, t, ms=(0, 3, 1, 2)):
        # chain order: op1 reads slots 0 and 3, then 1, then 2
        for m in ms:
            nc.sync.dma_start(wb_sb[:, m, t, ch], wb_d[m, t, ch])

    vts = {}

    def v_alloc(n, h):
        vts[(n, h)] = vpool.tile([128, CIT, UV, HALF], BF16, tag="vt", name="vt")

    def v_dma(n, h, t):
        nc.sync.dma_start(vts[(n, h)][:, t], v_d[n, h, t])

    aggs = {}

    def agg_alloc(n, ch):
        aggP = aggp_pool.tile([128, CIT, UV, 128], BF16, tag="aggP", name="aggP")
        aggN1 = aggn_pool.tile([128, CIT, 4, 128], BF16, tag="aggN1", name="aggN1")
        aggs[(n, ch)] = (aggP, aggN1)

    def agg_block(n, ch, t):
        # all on DVE: Pool rejects scalar_tensor_tensor at codegen. every
        # operand is a fully contiguous slab.
        s = n * M
        aggP, aggN1 = aggs[(n, ch)]
        o = aggP[:, t]
        nc.vector.scalar_tensor_tensor(o, wb_sb[:, 0, t, ch], pi_b[:, s : s + 1], wb_sb[:, 3, t, ch], op0=ALU.mult, op1=ALU.add)
        nc.vector.scalar_tensor_tensor(o, wb_sb[:, 1, t, ch], pi_b[:, s + 1 : s + 2], o, op0=ALU.mult, op1=ALU.add)
        nc.vector.scalar_tensor_tensor(o, wb_sb[:, 2, t, ch], pi_b[:, s + 2 : s + 3], o, op0=ALU.mult, op1=ALU.add)
        # negated u=1 planes (v-major: indices 1,5,9,13): the a=1 phase
        # accumulates q = -m1 + m2 + m3, drained with scale -1
        nc.vector.tensor_scalar_mul(aggN1[:, t], aggP[:, t, 1:16:4, :], -1.0)

    def phase(n, h, ct, a):
        vt = vts[(n, h)]
        aggP, aggN1 = aggs[(n, ct)]
        pt = psum.tile([128, 4, HALF], F32, tag="pt", name="pt")
        # plane order is v-major (p = v*4+u) so the head can stream V per
        # v-group; aggN1 holds only the u=1 planes (indexed by v)
        if a == 0:
            # t0 = m0 + m1 + m2
            terms = ((aggP, 0, lambda v: v * 4 + 0), (aggP, 1, lambda v: v * 4 + 1), (aggP, 2, lambda v: v * 4 + 2))
        else:
            # q = -m1 + m2 + m3 ; t1 = -q applied at drain time
            terms = ((aggN1, 1, lambda v: v), (aggP, 2, lambda v: v * 4 + 2), (aggP, 3, lambda v: v * 4 + 3))
        for v in range(4):
            i = 0
            for t in range(CIT):
                for wtile, u, plane in terms:
                    nc.tensor.matmul(
                        pt[:, v, :],
                        wtile[:, t, plane(v), :],
                        vt[:, t, v * 4 + u, :],
                        start=(i == 0),
                        stop=(i == 2 * len(terms) - 1),
                    )
                    i += 1
        # ScalarE drains the 4-bank PSUM block to SBUF f16 as FIVE planes
        # [t0, t1+bn, t2, -t2, -t3] (sign-flipped wholesale for a=1, where
        # the PSUM holds -t1 components). The negations live here because
        # DVE SUBTRACT has no fast-mode uop (~11x slower than ADD).
        sgn = 1.0 if a == 0 else -1.0
        bn = bnT[:, ct * NL + n : ct * NL + n + 1]
        tcp = tcp_pool.tile([128, 5, HALF], F16, tag="tcp", name="tcp")
        nc.scalar.activation(tcp[:, 0, :], pt[:, 0, :], AF.Copy, scale=sgn)
        nc.scalar.activation(tcp[:, 1, :], pt[:, 1, :], AF.Identity, bias=bn, scale=sgn)
        nc.scalar.activation(tcp[:, 2, :], pt[:, 2, :], AF.Copy, scale=sgn)
        nc.scalar.activation(tcp[:, 3, :], pt[:, 2, :], AF.Copy, scale=-sgn)
        nc.scalar.activation(tcp[:, 4, :], pt[:, 3, :], AF.Copy, scale=-sgn)
        # right A transform, pure 2x-mode ADDs on DVE:
        # y[b0] = (t0+t1)+t2 ; y[b1] = (t1+(-t2))+(-t3)
        ot = outp.tile([128, 2, HALF], F16, tag="ot", name="ot")
        s01 = scr_pool.tile([128, HALF], F16, tag="scr", name="scr")
        nc.vector.tensor_tensor(s01[:], tcp[:, 0, :], tcp[:, 1, :], op=ALU.add)
        nc.vector.tensor_tensor(ot[:, 0, :], s01[:], tcp[:, 2, :], op=ALU.add)
        d12 = scr_pool.tile([128, HALF], F16, tag="scr", name="scr")
        nc.vector.tensor_tensor(d12[:], tcp[:, 1, :], tcp[:, 3, :], op=ALU.add)
        nc.vector.tensor_tensor(ot[:, 1, :], d12[:], tcp[:, 4, :], op=ALU.add)
        nc.sync.dma_start(y_d[n, ct, :, a, :, h * HALF : (h + 1) * HALF], ot[:])

    # ---- head: the bytes gating the first matmuls lead, in consumption
    # order (the DMA ring completes in issue order): ch0 bank + V half 0,
    # then ch1 bank (phases 2-3), then V half 1 (phases 4-7). ----
    v_alloc(0, 0)
    v_alloc(0, 1)
    wb_dma(0, 0, ms=(0, 3))
    for g in range(2):
        for t in range(CIT):
            nc.sync.dma_start(vts[(0, 0)][:, t, 4 * g : 4 * g + 4, :], v_d[0, 0, t, :, 4 * g : 4 * g + 4, :])
    wb_dma(0, 0, ms=(1, 2))
    wb_dma(0, 1)
    for g in range(2, 4):
        for t in range(CIT):
            nc.sync.dma_start(vts[(0, 0)][:, t, 4 * g : 4 * g + 4, :], v_d[0, 0, t, :, 4 * g : 4 * g + 4, :])
    agg_alloc(0, 0)
    agg_block(0, 0, 0)
    agg_block(0, 0, 1)
    wb_dma(1, 0)
    wb_dma(1, 1)
    agg_alloc(0, 1)
    agg_block(0, 1, 0)
    agg_block(0, 1, 1)
    v_dma(0, 1, 0)
    v_dma(0, 1, 1)

    for n in range(NL):
        plan = [(h, ct, a) for h in range(2) for ct in range(COT) for a in range(2)]
        for i, (h, ct, a) in enumerate(plan):
            phase(n, h, ct, a)
            if n + 1 < NL:
                # interleave the next sample's aggregation and V prefetch at
                # the points where buffers free up (DVE and the sync queue
                # both execute in issue order)
                if i == 3:
                    v_alloc(n + 1, 0)
                    v_dma(n + 1, 0, 0)
                    v_dma(n + 1, 0, 1)
                    agg_alloc(n + 1, 0)
                    agg_block(n + 1, 0, 0)
                elif i == 4:
                    agg_block(n + 1, 0, 1)
                elif i == 5:
                    agg_alloc(n + 1, 1)
                    agg_block(n + 1, 1, 0)
                elif i == 6:
                    agg_block(n + 1, 1, 1)
                elif i == 7:
                    v_alloc(n + 1, 1)
                    v_dma(n + 1, 1, 0)
                    v_dma(n + 1, 1, 1)


def build_program():
    nc = bacc.Bacc("TRN2", target_bir_lowering=False, debug=False, num_devices=NCORES)
    with tile.TileContext(nc) as tc:
        with ExitStack() as ctx:
            _emit(ctx, tc)
    nc.compile()
    return nc


def _host_pi(x, w1, b1, w2, b2):
    pooled = x.mean(axis=(2, 3), dtype=np.float32)
    hmid = np.maximum(pooled @ np.asarray(w1, np.float32).T + np.asarray(b1, np.float32), 0)
    logits = hmid @ np.asarray(w2, np.float32).T + np.asarray(b2, np.float32)
    z = logits * TAU
    z = z - z.max(axis=1, keepdims=True)
    e = np.exp(z)
    return (e / e.sum(axis=1, keepdims=True)).astype(np.float32)


def _wino_input(x):
    """V[n, ci, uv, k, j] = (B^T d B) for 4x4 tiles of the padded input."""
    n, ci = x.shape[0], x.shape[1]
    xpad = np.zeros((n, ci, H + 2, W + 2), np.float32)
    xpad[:, :, 1 : H + 1, 1 : W + 1] = x
    e = xpad[:, :, :, 0::2]          # 33 even cols
    o = xpad[:, :, :, 1::2]          # 33 odd cols
    R = np.empty((4, n, ci, H + 2, G_T), np.float32)
    R[0] = e[:, :, :, :G_T] - e[:, :, :, 1:]
    R[1] = o[:, :, :, :G_T] + e[:, :, :, 1:]
    R[2] = e[:, :, :, 1:] - o[:, :, :, :G_T]
    R[3] = o[:, :, :, :G_T] - o[:, :, :, 1:]
    V = np.empty((n, ci, UV, G_T, G_T), np.float32)
    # plane order v-major: p = v*4 + u
    for v in range(4):
        er = R[v][:, :, 0::2, :]     # 33 even rows
        orr = R[v][:, :, 1::2, :]    # 33 odd rows
        V[:, :, v * 4 + 0] = er[:, :, :G_T] - er[:, :, 1:]
        V[:, :, v * 4 + 1] = orr[:, :, :G_T] + er[:, :, 1:]
        V[:, :, v * 4 + 2] = er[:, :, 1:] - orr[:, :, :G_T]
        V[:, :, v * 4 + 3] = orr[:, :, :G_T] - orr[:, :, 1:]
    return V


def prep_inputs(x, Wbank, Bbank, w1, b1, w2, b2):
    """Host-side layout prep. Returns per-core in_maps."""
    x = np.asarray(x, dtype=np.float32)
    Wbank = np.asarray(Wbank, dtype=np.float32)
    pi = _host_pi(x, w1, b1, w2, b2)                                   # N,M
    bn = pi @ np.asarray(Bbank, np.float32).T                          # N,CO

    # U = G W G^T per (co, m, ci); delta form over m
    G = np.array([[1, 0, 0], [0.5, 0.5, 0.5], [0.5, -0.5, 0.5], [0, 0, 1]], np.float32)
    # plane order v-major to match V: index [v, u]
    U = np.einsum("ua,omiab,vb->omivu", G, Wbank, G).astype(np.float32)  # Co,M,Ci,4v,4u
    Ud = np.empty_like(U)
    Ud[:, 3] = U[:, 3]
    for m in range(3):
        Ud[:, m] = U[:, m] - U[:, 3]
    # wb_d [m, cit, co-half, 128, uv, 128]
    wb = np.ascontiguousarray(
        Ud.reshape(COT, 128, M, CIT, 128, UV).transpose(2, 3, 0, 4, 5, 1)
    ).astype(BF16_NP)

    # V -> v_d [NL_core..., 2, cit, 128, uv, 512]
    V = _wino_input(x)                                                 # N,CI,16,32,32
    Vr = V.reshape(N, CIT, 128, UV, GRID)
    v_all = np.ascontiguousarray(
        Vr.reshape(N, CIT, 128, UV, 2, HALF).transpose(0, 4, 1, 2, 3, 5)
    ).astype(BF16_NP)                                                  # N,2,CIT,128,UV,HALF

    in_maps = []
    for c in range(NCORES):
        sl = slice(c * NL, (c + 1) * NL)
        cst = np.zeros((128, 24), dtype=np.float32)
        cst[:, 0:16] = np.broadcast_to(pi[sl].reshape(1, NL * M), (128, NL * M))
        cst[:, 16:24] = bn[sl].reshape(NL, COT, 128).transpose(2, 1, 0).reshape(128, COT * NL)
        in_maps.append({"vt": np.ascontiguousarray(v_all[sl]), "wb": wb, "cst": cst})
    return in_maps


def kernel(x, Wbank, Bbank, w1, b1, w2, b2):
    x = np.asarray(x, dtype=np.float32)
    in_maps = prep_inputs(x, Wbank, Bbank, w1, b1, w2, b2)
    if "nc" not in _CACHE:
        _CACHE["nc"] = build_program()
    res = bass_utils.run_bass_kernel_spmd(_CACHE["nc"], in_maps, core_ids=list(range(NCORES)))
    outs = []
    for r in res.results:
        y = r["y"].astype(np.float32)                                  # NL,COT,128,2,2,GRID
        y = y.reshape(NL, COT, 128, 2, 2, G_T, G_T)
        y = y.transpose(0, 1, 2, 5, 3, 6, 4).reshape(NL, CO, H, W)
        outs.append(y)
    return np.concatenate(outs, axis=0)
